# revision 1
# baseline (speedup 1.0000x reference)
"""Trainium2 Bass kernel for nn_MultiHeadAttention (B=8, S=2048, D=512, H=8).

Sharding: data-parallel over batch; core i computes batch element i end-to-end.
Key compaction: masked key positions contribute exactly zero after softmax, so
the host gathers only unmasked key rows (padded to a multiple of 128 with
entries whose mask bias of -30000 makes exp() underflow to zero).
"""

import sys
import types

for _p in ("/opt/trn_rl_repo", "/root/.axon_site"):
    if _p not in sys.path:
        sys.path.insert(0, _p)

import numpy as np

B, S, DIM, H, HD = 8, 2048, 512, 8, 64
NPART = 128
PAIRS = H // 2  # head pairs (2 heads share a 128-partition tile)
ST = S // NPART  # 16 query/row tiles
QB = 512  # query block width for attention
NQB = S // QB  # 4
MASK_BIAS = -30000.0

TRACE = False
TRACE_KWARGS = {}
LAST_RESULT = None
_PROG_CACHE = {}


def _host_prep(x, mask, Wqkv, bqkv, Wout, bout, ln_g, ln_b):
    x = np.ascontiguousarray(x, dtype=np.float32)
    mask = np.asarray(mask).astype(bool)
    Wqkv = np.asarray(Wqkv, dtype=np.float32)
    bqkv = np.asarray(bqkv, dtype=np.float32)
    Wout = np.asarray(Wout, dtype=np.float32)
    bout = np.asarray(bout, dtype=np.float32)

    # torch-reshape-interleaved channel indices: j = 192*h + 3*d + c
    d_idx = np.arange(HD)
    perm_q = np.concatenate([192 * h + 3 * d_idx for h in range(H)])
    perm_k = perm_q + 1
    perm_v = perm_q + 2

    wqT = np.ascontiguousarray(Wqkv[perm_q, :].T)  # [512 d, 512 ch(h-major)]
    wkT = np.ascontiguousarray(Wqkv[perm_k, :].T)
    wvT = np.ascontiguousarray(Wqkv[perm_v, :].T)
    bq = np.ascontiguousarray(bqkv[perm_q].reshape(PAIRS, NPART))
    bk = np.ascontiguousarray(bqkv[perm_k].reshape(PAIRS, NPART))
    bv = np.ascontiguousarray(bqkv[perm_v].reshape(1, DIM))
    # Wout columns are already (h, d)-ordered; wo8[d, h, e] for per-head lhsT
    wo8 = np.ascontiguousarray(Wout.T.reshape(H, HD, DIM).transpose(1, 0, 2))
    bout_r = np.ascontiguousarray(bout.reshape(1, DIM))

    genln = not (np.allclose(ln_g, 1.0) and np.allclose(ln_b, 0.0))

    keep = [np.flatnonzero(~mask[b]) for b in range(B)]
    nk_max = max(len(k) for k in keep)
    NK = max(NPART, ((nk_max + NPART - 1) // NPART) * NPART)
    NKT = NK // NPART

    per_core = []
    for b in range(B):
        kidx = np.zeros(NK, dtype=np.int64)
        kidx[: len(keep[b])] = keep[b]
        mb = np.full(NK, MASK_BIAS, dtype=np.float32)
        mb[: len(keep[b])] = 0.0
        per_core.append(
            dict(
                xq=np.ascontiguousarray(x[b].T),  # [512, 2048]
                xk=np.ascontiguousarray(x[b][kidx, :].T),  # [512, NK]
                mb=np.ascontiguousarray(mb.reshape(NKT, NPART).T),  # [128, NKT]
            )
        )

    shared = dict(
        wqT=wqT, wkT=wkT, wvT=wvT, bq=bq, bk=bk, bv=bv,
        wo8=wo8, bout=bout_r,
        ones=np.ones((1, NPART), dtype=np.float32),
        vones=np.ones((NPART, NKT, H), dtype=np.float32),
    )
    if genln:
        shared["lng"] = np.ascontiguousarray(np.asarray(ln_g, np.float32).reshape(1, DIM))
        shared["lnb"] = np.ascontiguousarray(np.asarray(ln_b, np.float32).reshape(1, DIM))
    return per_core, shared, NKT, genln


def _build_program(NKT, genln):
    import concourse.bass as bass
    import concourse.tile as tile
    from concourse import mybir, bacc

    F32 = mybir.dt.float32
    F32R = mybir.dt.float32r
    AF = mybir.ActivationFunctionType
    OP = mybir.AluOpType
    NK = NKT * NPART

    nc = bacc.Bacc("TRN2", target_bir_lowering=False, debug=False)

    d_xq = nc.dram_tensor("xq", [DIM, S], F32R, kind="ExternalInput").ap()
    d_xk = nc.dram_tensor("xk", [DIM, NK], F32R, kind="ExternalInput").ap()
    d_mb = nc.dram_tensor("mb", [NPART, NKT], F32, kind="ExternalInput").ap()
    d_wqT = nc.dram_tensor("wqT", [DIM, DIM], F32R, kind="ExternalInput").ap()
    d_wkT = nc.dram_tensor("wkT", [DIM, DIM], F32R, kind="ExternalInput").ap()
    d_wvT = nc.dram_tensor("wvT", [DIM, DIM], F32R, kind="ExternalInput").ap()
    d_bq = nc.dram_tensor("bq", [PAIRS, NPART], F32, kind="ExternalInput").ap()
    d_bk = nc.dram_tensor("bk", [PAIRS, NPART], F32, kind="ExternalInput").ap()
    d_bv = nc.dram_tensor("bv", [1, DIM], F32R, kind="ExternalInput").ap()
    d_wo8 = nc.dram_tensor("wo8", [HD, H, DIM], F32R, kind="ExternalInput").ap()
    d_bout = nc.dram_tensor("bout", [1, DIM], F32R, kind="ExternalInput").ap()
    d_ones = nc.dram_tensor("ones", [1, NPART], F32R, kind="ExternalInput").ap()
    d_vones = nc.dram_tensor("vones", [NPART, NKT, H], F32R, kind="ExternalInput").ap()
    if genln:
        d_lng = nc.dram_tensor("lng", [1, DIM], F32, kind="ExternalInput").ap()
        d_lnb = nc.dram_tensor("lnb", [1, DIM], F32, kind="ExternalInput").ap()
    d_out = nc.dram_tensor("out", [S, DIM], F32, kind="ExternalOutput").ap()
    d_oraw = nc.dram_tensor("oraw", [S, DIM], F32)  # pre-LN spill

    with tile.TileContext(nc) as tc:
        with (
            tc.tile_pool(name="persist", bufs=1) as pp,
            tc.tile_pool(name="ps_s", bufs=2, space="PSUM") as ps_s,
            tc.tile_pool(name="ps_av", bufs=4, space="PSUM") as ps_av,
            tc.tile_pool(name="tmp", bufs=3) as tmp,
        ):
            # ---- persistent tiles ----
            qall = pp.tile([NPART, PAIRS, S], F32R)
            kall = pp.tile([NPART, PAIRS, NK], F32R)
            v_sb = pp.tile([NPART, NKT, H * (HD + 1)], F32R)
            mb_sb = pp.tile([NPART, NKT], F32)
            bq_sb = pp.tile([NPART, PAIRS], F32)
            bk_sb = pp.tile([NPART, PAIRS], F32)
            bv_sb = pp.tile([1, DIM], F32R)
            bout_sb = pp.tile([1, DIM], F32R)
            ones1 = pp.tile([1, NPART], F32R)
            wo8_sb = pp.tile([HD, H, DIM], F32R)
            eps_sb = pp.tile([NPART, 1], F32)
            stats_sb = pp.tile([NPART, ST, 6], F32)
            mv_sb = pp.tile([NPART, ST, 2], F32)
            sd_sb = pp.tile([NPART, ST], F32)
            rsig_sb = pp.tile([NPART, ST], F32)

            nc.sync.dma_start(out=mb_sb, in_=d_mb)
            for p in range(PAIRS):
                nc.sync.dma_start(out=bq_sb[:, p : p + 1], in_=d_bq[p : p + 1, :].rearrange("a b -> b a"))
                nc.sync.dma_start(out=bk_sb[:, p : p + 1], in_=d_bk[p : p + 1, :].rearrange("a b -> b a"))
            nc.sync.dma_start(out=bv_sb, in_=d_bv)
            nc.sync.dma_start(out=bout_sb, in_=d_bout)
            nc.sync.dma_start(out=ones1, in_=d_ones)
            nc.sync.dma_start(out=wo8_sb, in_=d_wo8)
            nc.vector.memset(eps_sb, 1e-5)
            vs = v_sb.rearrange("p t (h c) -> p t h c", c=HD + 1)
            nc.sync.dma_start(out=vs[:, :, :, HD : HD + 1], in_=d_vones)
            if genln:
                g_row = pp.tile([1, DIM], F32)
                b_row = pp.tile([1, DIM], F32)
                gbc = pp.tile([NPART, DIM], F32)
                bbc = pp.tile([NPART, DIM], F32)
                nc.sync.dma_start(out=g_row, in_=d_lng)
                nc.sync.dma_start(out=b_row, in_=d_lnb)
                nc.gpsimd.partition_broadcast(gbc, g_row)
                nc.gpsimd.partition_broadcast(bbc, b_row)

            # ================= phase A1: Q/K projections =================
            with (
                tc.tile_pool(name="pA1w", bufs=1) as pA1w,
                tc.tile_pool(name="pAxq", bufs=1) as pAxq,
                tc.tile_pool(name="pAxk", bufs=1) as pAxk,
            ):
                wq_sb = pA1w.tile([NPART, 4, DIM], F32R)
                wk_sb = pA1w.tile([NPART, 4, DIM], F32R)
                xq_sb = pAxq.tile([NPART, 4, S], F32R)
                xk_sb = pAxk.tile([NPART, 4, NK], F32R)
                wq_v = d_wqT.rearrange("(kt p) c -> p kt c", p=NPART)
                wk_v = d_wkT.rearrange("(kt p) c -> p kt c", p=NPART)
                xq_v = d_xq.rearrange("(kt p) s -> p kt s", p=NPART)
                xk_v = d_xk.rearrange("(kt p) s -> p kt s", p=NPART)
                for kt in range(4):
                    nc.sync.dma_start(out=wq_sb[:, kt, :], in_=wq_v[:, kt, :])
                    nc.sync.dma_start(out=wk_sb[:, kt, :], in_=wk_v[:, kt, :])
                    nc.sync.dma_start(out=xq_sb[:, kt, :], in_=xq_v[:, kt, :])
                    nc.sync.dma_start(out=xk_sb[:, kt, :], in_=xk_v[:, kt, :])

                for p in range(PAIRS):
                    for n0 in range(0, S, QB):
                        ps = ps_s.tile([NPART, 2 * QB], F32, tag="s")
                        for kt in range(4):
                            nc.tensor.matmul(
                                out=ps[:, 0:QB],
                                lhsT=wq_sb[:, kt, p * NPART : (p + 1) * NPART],
                                rhs=xq_sb[:, kt, n0 : n0 + QB],
                                start=(kt == 0), stop=(kt == 3),
                            )
                        nc.vector.tensor_scalar_add(
                            out=qall[:, p, n0 : n0 + QB], in0=ps[:, 0:QB],
                            scalar1=bq_sb[:, p : p + 1],
                        )
                    for n0 in range(0, NK, QB):
                        n1 = min(n0 + QB, NK)
                        ps = ps_s.tile([NPART, 2 * QB], F32, tag="s")
                        for kt in range(4):
                            nc.tensor.matmul(
                                out=ps[:, 0 : n1 - n0],
                                lhsT=wk_sb[:, kt, p * NPART : (p + 1) * NPART],
                                rhs=xk_sb[:, kt, n0:n1],
                                start=(kt == 0), stop=(kt == 3),
                            )
                        nc.vector.tensor_scalar_add(
                            out=kall[:, p, n0:n1], in0=ps[:, 0 : n1 - n0],
                            scalar1=bk_sb[:, p : p + 1],
                        )

                # ============= phase A2: V projection + ELU =============
                with tc.tile_pool(name="pA2w", bufs=1) as pA2w:
                    wv_sb = pA2w.tile([NPART, 4, DIM], F32R)
                    wv_v = d_wvT.rearrange("(kt p) c -> p kt c", p=NPART)
                    for kt in range(4):
                        nc.sync.dma_start(out=wv_sb[:, kt, :], in_=wv_v[:, kt, :])
                    for st in range(NKT):
                        ps = ps_s.tile([NPART, 2 * QB], F32, tag="s")
                        pv = ps[:, 0:DIM]
                        for kt in range(4):
                            nc.tensor.matmul(
                                out=pv,
                                lhsT=xk_sb[:, kt, st * NPART : (st + 1) * NPART],
                                rhs=wv_sb[:, kt, :],
                                start=(kt == 0), stop=False,
                            )
                        nc.tensor.matmul(out=pv, lhsT=ones1, rhs=bv_sb, start=False, stop=True)
                        # elu(x) = exp(min(x,0)) + max(x,0) - 1
                        tmin = tmp.tile([NPART, DIM], F32, tag="t0")
                        nc.vector.tensor_scalar_min(out=tmin, in0=pv, scalar1=0.0)
                        te = tmp.tile([NPART, DIM], F32, tag="t1")
                        nc.scalar.activation(out=te, in_=tmin, func=AF.Exp)
                        tr = tmp.tile([NPART, DIM], F32, tag="t2")
                        nc.vector.tensor_scalar(
                            out=tr, in0=pv, scalar1=0.0, scalar2=-1.0,
                            op0=OP.max, op1=OP.add,
                        )
                        nc.vector.tensor_tensor(
                            out=vs[:, st, :, 0:HD],
                            in0=te.rearrange("p (h c) -> p h c", c=HD),
                            in1=tr.rearrange("p (h c) -> p h c", c=HD),
                            op=OP.add,
                        )

            # ================= phase B: attention + out-proj =================
            with (
                tc.tile_pool(name="pT", bufs=3) as pTp,
                tc.tile_pool(name="oTq", bufs=2) as oTq,
                tc.tile_pool(name="rdp", bufs=2) as rdp,
                tc.tile_pool(name="rdbcp", bufs=2) as rdbcp,
                tc.tile_pool(name="co", bufs=2) as cop,
            ):
                for q in range(NQB):
                    q0 = q * QB
                    oT_q = oTq.tile([HD, H, QB], F32R, tag="oT")
                    for pr in range(PAIRS):
                        hA, hB = 2 * pr, 2 * pr + 1
                        avA = ps_av.tile([HD + 1, QB], F32, tag="av")
                        avB = ps_av.tile([HD + 1, QB], F32, tag="av")
                        for kt in range(NKT):
                            sp = ps_s.tile([NPART, 2 * QB], F32, tag="s")
                            nc.tensor.matmul(
                                out=sp[:, 0:QB],
                                lhsT=kall[0:HD, pr, kt * NPART : (kt + 1) * NPART],
                                rhs=qall[0:HD, pr, q0 : q0 + QB],
                                start=True, stop=True,
                            )
                            nc.tensor.matmul(
                                out=sp[:, QB : 2 * QB],
                                lhsT=kall[HD:NPART, pr, kt * NPART : (kt + 1) * NPART],
                                rhs=qall[HD:NPART, pr, q0 : q0 + QB],
                                start=True, stop=True,
                            )
                            pt = pTp.tile([NPART, 2 * QB], F32R, tag="pt")
                            nc.scalar.activation(
                                out=pt, in_=sp, func=AF.Exp,
                                bias=mb_sb[:, kt : kt + 1], scale=1.0 / 8.0,
                            )
                            nc.tensor.matmul(
                                out=avA,
                                lhsT=vs[:, kt, hA, :],
                                rhs=pt[:, 0:QB],
                                start=(kt == 0), stop=(kt == NKT - 1),
                            )
                            nc.tensor.matmul(
                                out=avB,
                                lhsT=vs[:, kt, hB, :],
                                rhs=pt[:, QB : 2 * QB],
                                start=(kt == 0), stop=(kt == NKT - 1),
                            )
                        for head, av in ((hA, avA), (hB, avB)):
                            rd65 = rdp.tile([HD + 1, QB], F32, tag="rd")
                            nc.vector.reciprocal(out=rd65[HD : HD + 1, :], in_=av[HD : HD + 1, :])
                            rd0 = rdp.tile([1, QB], F32, tag="rd0")
                            nc.vector.tensor_copy(out=rd0, in_=rd65[HD : HD + 1, :])
                            rdbc = rdbcp.tile([HD, QB], F32, tag="rdbc")
                            nc.gpsimd.partition_broadcast(rdbc, rd0)
                            nc.vector.tensor_tensor(
                                out=oT_q[:, head, :], in0=av[0:HD, :], in1=rdbc, op=OP.mult,
                            )
                    # ---- phase C1 for this q block: out-proj + stats + spill ----
                    for j in range(4):
                        st = q * 4 + j
                        po_t = ps_s.tile([NPART, 2 * QB], F32, tag="s")
                        po = po_t[:, 0:DIM]
                        for h in range(H):
                            nc.tensor.matmul(
                                out=po,
                                lhsT=oT_q[:, h, j * NPART : (j + 1) * NPART],
                                rhs=wo8_sb[:, h, :],
                                start=(h == 0), stop=False,
                            )
                        nc.tensor.matmul(out=po, lhsT=ones1, rhs=bout_sb, start=False, stop=True)
                        nc.vector.bn_stats(out=stats_sb[:, st, :], in_=po)
                        oc = cop.tile([NPART, DIM], F32, tag="oc")
                        nc.vector.tensor_copy(out=oc, in_=po)
                        nc.sync.dma_start(
                            out=d_oraw[st * NPART : (st + 1) * NPART, :], in_=oc
                        )

                # ================= phase C2: LN stats =================
                for st in range(ST):
                    nc.vector.bn_aggr(out=mv_sb[:, st, :], in_=stats_sb[:, st, :])
                nc.scalar.activation(
                    out=sd_sb, in_=mv_sb[:, :, 1], func=AF.Sqrt, bias=eps_sb,
                )
                nc.vector.reciprocal(out=rsig_sb, in_=sd_sb)

                # ================= phase C3: LN apply + ELU + store =================
                for st in range(ST):
                    ld = cop.tile([NPART, DIM], F32, tag="ld")
                    nc.sync.dma_start(
                        out=ld, in_=d_oraw[st * NPART : (st + 1) * NPART, :]
                    )
                    y = cop.tile([NPART, DIM], F32, tag="y")
                    nc.vector.tensor_scalar(
                        out=y, in0=ld,
                        scalar1=mv_sb[:, st, 0:1], scalar2=rsig_sb[:, st : st + 1],
                        op0=OP.subtract, op1=OP.mult,
                    )
                    if genln:
                        nc.vector.tensor_tensor(out=y, in0=y, in1=gbc, op=OP.mult)
                        nc.vector.tensor_tensor(out=y, in0=y, in1=bbc, op=OP.add)
                    tmin = tmp.tile([NPART, DIM], F32, tag="t0")
                    nc.vector.tensor_scalar_min(out=tmin, in0=y, scalar1=0.0)
                    te = tmp.tile([NPART, DIM], F32, tag="t1")
                    nc.scalar.activation(out=te, in_=tmin, func=AF.Exp)
                    tr = tmp.tile([NPART, DIM], F32, tag="t2")
                    nc.vector.tensor_scalar(
                        out=tr, in0=y, scalar1=0.0, scalar2=-1.0, op0=OP.max, op1=OP.add,
                    )
                    fin = cop.tile([NPART, DIM], F32, tag="fin")
                    nc.vector.tensor_tensor(out=fin, in0=te, in1=tr, op=OP.add)
                    nc.sync.dma_start(
                        out=d_out[st * NPART : (st + 1) * NPART, :], in_=fin
                    )

    nc.compile()
    return nc


def _install_trace_hook():
    try:
        import antenv  # noqa: F401
        from trn_agent_boot.trn_boot import _ntff_profile_via_ctypes

        hook = _ntff_profile_via_ctypes("/opt/axon/libaxon_pjrt.so")
        mod = types.ModuleType("antenv.axon_hooks")
        mod.get_axon_ntff_profile_hook = lambda: hook
        sys.modules["antenv.axon_hooks"] = mod
    except Exception:
        pass


def kernel(x, mask, Wqkv, bqkv, Wout, bout, ln_g, ln_b):
    global LAST_RESULT
    from concourse.bass_utils import run_bass_kernel_spmd

    per_core, shared, NKT, genln = _host_prep(
        x, mask, Wqkv, bqkv, Wout, bout, ln_g, ln_b
    )
    key = (NKT, genln)
    if key not in _PROG_CACHE:
        _PROG_CACHE[key] = _build_program(NKT, genln)
    nc = _PROG_CACHE[key]

    in_maps = [{**shared, **pc} for pc in per_core]
    if TRACE:
        _install_trace_hook()
    res = run_bass_kernel_spmd(
        nc, in_maps, list(range(B)), trace=TRACE, **TRACE_KWARGS
    )
    LAST_RESULT = res
    out = np.stack([res.results[i]["out"] for i in range(B)], axis=0)
    return out.astype(np.float32)


# revision 3
# speedup vs baseline: 1.1388x; 1.1388x over previous
"""Trainium2 Bass kernel for nn_MultiHeadAttention (B=8, S=2048, D=512, H=8).

Sharding: data-parallel over batch; core i computes batch element i end-to-end.
Key compaction: masked key positions contribute exactly zero after softmax, so
the host gathers only unmasked key rows (padded to a multiple of 128 with
entries whose mask bias of -30000 makes exp() underflow to zero).
"""

import sys
import types

for _p in ("/opt/trn_rl_repo", "/root/.axon_site"):
    if _p not in sys.path:
        sys.path.insert(0, _p)

import numpy as np

B, S, DIM, H, HD = 8, 2048, 512, 8, 64
NPART = 128
PAIRS = H // 2  # head pairs (2 heads share a 128-partition tile)
ST = S // NPART  # 16 query/row tiles
QB = 512  # query block width for attention
NQB = S // QB  # 4
MASK_BIAS = -30000.0

TRACE = False
TRACE_KWARGS = {}
LAST_RESULT = None
_PROG_CACHE = {}


def _host_prep(x, mask, Wqkv, bqkv, Wout, bout, ln_g, ln_b):
    x = np.ascontiguousarray(x, dtype=np.float32)
    mask = np.asarray(mask).astype(bool)
    Wqkv = np.asarray(Wqkv, dtype=np.float32)
    bqkv = np.asarray(bqkv, dtype=np.float32)
    Wout = np.asarray(Wout, dtype=np.float32)
    bout = np.asarray(bout, dtype=np.float32)

    # torch-reshape-interleaved channel indices: j = 192*h + 3*d + c
    d_idx = np.arange(HD)
    perm_q = np.concatenate([192 * h + 3 * d_idx for h in range(H)])
    perm_k = perm_q + 1
    perm_v = perm_q + 2

    wqT = np.ascontiguousarray(Wqkv[perm_q, :].T)  # [512 d, 512 ch(h-major)]
    wkT = np.ascontiguousarray(Wqkv[perm_k, :].T)
    wvT = np.ascontiguousarray(Wqkv[perm_v, :].T)
    bq = np.ascontiguousarray(bqkv[perm_q].reshape(PAIRS, NPART))
    bk = np.ascontiguousarray(bqkv[perm_k].reshape(PAIRS, NPART))
    bv = np.ascontiguousarray(bqkv[perm_v].reshape(1, DIM))
    # Wout columns are already (h, d)-ordered; head-pair-major rows for out-proj rhs
    woT = np.ascontiguousarray(Wout.T)  # [512 c(h-major), 512 e]
    bout_r = np.ascontiguousarray(bout.reshape(1, DIM))

    genln = not (np.allclose(ln_g, 1.0) and np.allclose(ln_b, 0.0))

    keep = [np.flatnonzero(~mask[b]) for b in range(B)]
    nk_max = max(len(k) for k in keep)
    NK = max(NPART, ((nk_max + NPART - 1) // NPART) * NPART)
    NKT = NK // NPART

    per_core = []
    for b in range(B):
        kidx = np.zeros(NK, dtype=np.int64)
        kidx[: len(keep[b])] = keep[b]
        mb = np.full(NK, MASK_BIAS, dtype=np.float32)
        mb[: len(keep[b])] = 0.0
        per_core.append(
            dict(
                xq=np.ascontiguousarray(x[b].T),  # [512, 2048]
                xk=np.ascontiguousarray(x[b][kidx, :].T),  # [512, NK]
                mb=np.ascontiguousarray(mb.reshape(NKT, NPART).T),  # [128, NKT]
            )
        )

    shared = dict(
        wqT=wqT, wkT=wkT, wvT=wvT, bq=bq, bk=bk, bv=bv,
        woT=woT, bout=bout_r,
        ones=np.ones((1, NPART), dtype=np.float32),
        vones=np.ones((NPART, NKT, H), dtype=np.float32),
    )
    if genln:
        shared["lng"] = np.ascontiguousarray(np.asarray(ln_g, np.float32).reshape(1, DIM))
        shared["lnb"] = np.ascontiguousarray(np.asarray(ln_b, np.float32).reshape(1, DIM))
    return per_core, shared, NKT, genln


def _build_program(NKT, genln):
    import concourse.bass as bass
    import concourse.tile as tile
    from concourse import mybir, bacc

    F32 = mybir.dt.float32
    F32R = mybir.dt.float32r
    AF = mybir.ActivationFunctionType
    OP = mybir.AluOpType
    NK = NKT * NPART

    nc = bacc.Bacc("TRN2", target_bir_lowering=False, debug=False)

    d_xq = nc.dram_tensor("xq", [DIM, S], F32R, kind="ExternalInput").ap()
    d_xk = nc.dram_tensor("xk", [DIM, NK], F32R, kind="ExternalInput").ap()
    d_mb = nc.dram_tensor("mb", [NPART, NKT], F32, kind="ExternalInput").ap()
    d_wqT = nc.dram_tensor("wqT", [DIM, DIM], F32R, kind="ExternalInput").ap()
    d_wkT = nc.dram_tensor("wkT", [DIM, DIM], F32R, kind="ExternalInput").ap()
    d_wvT = nc.dram_tensor("wvT", [DIM, DIM], F32R, kind="ExternalInput").ap()
    d_bq = nc.dram_tensor("bq", [PAIRS, NPART], F32, kind="ExternalInput").ap()
    d_bk = nc.dram_tensor("bk", [PAIRS, NPART], F32, kind="ExternalInput").ap()
    d_bv = nc.dram_tensor("bv", [1, DIM], F32R, kind="ExternalInput").ap()
    d_woT = nc.dram_tensor("woT", [DIM, DIM], F32R, kind="ExternalInput").ap()
    d_bout = nc.dram_tensor("bout", [1, DIM], F32R, kind="ExternalInput").ap()
    d_ones = nc.dram_tensor("ones", [1, NPART], F32R, kind="ExternalInput").ap()
    d_vones = nc.dram_tensor("vones", [NPART, NKT, H], F32R, kind="ExternalInput").ap()
    if genln:
        d_lng = nc.dram_tensor("lng", [1, DIM], F32, kind="ExternalInput").ap()
        d_lnb = nc.dram_tensor("lnb", [1, DIM], F32, kind="ExternalInput").ap()
    d_out = nc.dram_tensor("out", [S, DIM], F32, kind="ExternalOutput").ap()
    d_oraw = nc.dram_tensor("oraw", [S, DIM], F32)  # pre-LN spill

    with tile.TileContext(nc) as tc:
        with (
            tc.tile_pool(name="persist", bufs=1) as pp,
            tc.tile_pool(name="ps_s", bufs=2, space="PSUM") as ps_s,
            tc.tile_pool(name="ps_av", bufs=4, space="PSUM") as ps_av,
            tc.tile_pool(name="tmp", bufs=3) as tmp,
        ):
            # ---- persistent tiles ----
            qall = pp.tile([NPART, PAIRS, S], F32R)
            kall = pp.tile([NPART, PAIRS, NK], F32R)
            v_sb = pp.tile([NPART, NKT, H * (HD + 1)], F32R)
            mb_sb = pp.tile([NPART, NKT], F32)
            bq_sb = pp.tile([NPART, PAIRS], F32)
            bk_sb = pp.tile([NPART, PAIRS], F32)
            bv_sb = pp.tile([1, DIM], F32R)
            bout_sb = pp.tile([1, DIM], F32R)
            ones1 = pp.tile([1, NPART], F32R)
            wo_sb = pp.tile([NPART, PAIRS, DIM], F32R)
            eps_sb = pp.tile([NPART, 1], F32)
            stats_sb = pp.tile([NPART, ST, 6], F32)
            mv_sb = pp.tile([NPART, ST, 2], F32)
            sd_sb = pp.tile([NPART, ST], F32)
            rsig_sb = pp.tile([NPART, ST], F32)

            nc.sync.dma_start(out=mb_sb, in_=d_mb)
            for p in range(PAIRS):
                nc.sync.dma_start(out=bq_sb[:, p : p + 1], in_=d_bq[p : p + 1, :].rearrange("a b -> b a"))
                nc.sync.dma_start(out=bk_sb[:, p : p + 1], in_=d_bk[p : p + 1, :].rearrange("a b -> b a"))
            nc.sync.dma_start(out=bv_sb, in_=d_bv)
            nc.sync.dma_start(out=bout_sb, in_=d_bout)
            nc.sync.dma_start(out=ones1, in_=d_ones)
            nc.sync.dma_start(out=wo_sb, in_=d_woT.rearrange("(pr p) e -> p pr e", p=NPART))
            nc.vector.memset(eps_sb, 1e-5)
            vs = v_sb.rearrange("p t (h c) -> p t h c", c=HD + 1)
            nc.sync.dma_start(out=vs[:, :, :, HD : HD + 1], in_=d_vones)
            if genln:
                g_row = pp.tile([1, DIM], F32)
                b_row = pp.tile([1, DIM], F32)
                gbc = pp.tile([NPART, DIM], F32)
                bbc = pp.tile([NPART, DIM], F32)
                nc.sync.dma_start(out=g_row, in_=d_lng)
                nc.sync.dma_start(out=b_row, in_=d_lnb)
                nc.gpsimd.partition_broadcast(gbc, g_row)
                nc.gpsimd.partition_broadcast(bbc, b_row)

            # ================= phase A1: Q/K projections =================
            with (
                tc.tile_pool(name="pA1w", bufs=1) as pA1w,
                tc.tile_pool(name="pAxq", bufs=1) as pAxq,
                tc.tile_pool(name="pAxk", bufs=1) as pAxk,
            ):
                wq_sb = pA1w.tile([NPART, 4, DIM], F32R)
                wk_sb = pA1w.tile([NPART, 4, DIM], F32R)
                xq_sb = pAxq.tile([NPART, 4, S], F32R)
                xk_sb = pAxk.tile([NPART, 4, NK], F32R)
                wq_v = d_wqT.rearrange("(kt p) c -> p kt c", p=NPART)
                wk_v = d_wkT.rearrange("(kt p) c -> p kt c", p=NPART)
                xq_v = d_xq.rearrange("(kt p) s -> p kt s", p=NPART)
                xk_v = d_xk.rearrange("(kt p) s -> p kt s", p=NPART)
                for kt in range(4):
                    nc.sync.dma_start(out=wq_sb[:, kt, :], in_=wq_v[:, kt, :])
                    nc.sync.dma_start(out=wk_sb[:, kt, :], in_=wk_v[:, kt, :])
                    nc.sync.dma_start(out=xq_sb[:, kt, :], in_=xq_v[:, kt, :])
                    nc.sync.dma_start(out=xk_sb[:, kt, :], in_=xk_v[:, kt, :])

                for p in range(PAIRS):
                    for n0 in range(0, S, QB):
                        ps = ps_s.tile([NPART, 2 * QB], F32, tag="s")
                        for kt in range(4):
                            nc.tensor.matmul(
                                out=ps[:, 0:QB],
                                lhsT=wq_sb[:, kt, p * NPART : (p + 1) * NPART],
                                rhs=xq_sb[:, kt, n0 : n0 + QB],
                                start=(kt == 0), stop=(kt == 3),
                            )
                        nc.vector.tensor_scalar_add(
                            out=qall[:, p, n0 : n0 + QB], in0=ps[:, 0:QB],
                            scalar1=bq_sb[:, p : p + 1],
                        )
                    for n0 in range(0, NK, QB):
                        n1 = min(n0 + QB, NK)
                        ps = ps_s.tile([NPART, 2 * QB], F32, tag="s")
                        for kt in range(4):
                            nc.tensor.matmul(
                                out=ps[:, 0 : n1 - n0],
                                lhsT=wk_sb[:, kt, p * NPART : (p + 1) * NPART],
                                rhs=xk_sb[:, kt, n0:n1],
                                start=(kt == 0), stop=(kt == 3),
                            )
                        nc.vector.tensor_scalar_add(
                            out=kall[:, p, n0:n1], in0=ps[:, 0 : n1 - n0],
                            scalar1=bk_sb[:, p : p + 1],
                        )

                # ============= phase A2: V projection + ELU =============
                with tc.tile_pool(name="pA2w", bufs=1) as pA2w:
                    wv_sb = pA2w.tile([NPART, 4, DIM], F32R)
                    wv_v = d_wvT.rearrange("(kt p) c -> p kt c", p=NPART)
                    for kt in range(4):
                        nc.sync.dma_start(out=wv_sb[:, kt, :], in_=wv_v[:, kt, :])
                    for st in range(NKT):
                        ps = ps_s.tile([NPART, 2 * QB], F32, tag="s")
                        pv = ps[:, 0:DIM]
                        for kt in range(4):
                            nc.tensor.matmul(
                                out=pv,
                                lhsT=xk_sb[:, kt, st * NPART : (st + 1) * NPART],
                                rhs=wv_sb[:, kt, :],
                                start=(kt == 0), stop=False,
                            )
                        nc.tensor.matmul(out=pv, lhsT=ones1, rhs=bv_sb, start=False, stop=True)
                        # elu(x) = exp(min(x,0)) + max(x,0) - 1
                        tmin = tmp.tile([NPART, DIM], F32, tag="t0")
                        nc.vector.tensor_scalar_min(out=tmin, in0=pv, scalar1=0.0)
                        te = tmp.tile([NPART, DIM], F32, tag="t1")
                        nc.scalar.activation(out=te, in_=tmin, func=AF.Exp)
                        tr = tmp.tile([NPART, DIM], F32, tag="t2")
                        nc.vector.tensor_scalar(
                            out=tr, in0=pv, scalar1=0.0, scalar2=-1.0,
                            op0=OP.max, op1=OP.add,
                        )
                        nc.vector.tensor_tensor(
                            out=vs[:, st, :, 0:HD],
                            in0=te.rearrange("p (h c) -> p h c", c=HD),
                            in1=tr.rearrange("p (h c) -> p h c", c=HD),
                            op=OP.add,
                        )

            # ================= phase B: attention + out-proj =================
            with (
                tc.tile_pool(name="pT", bufs=3) as pTp,
                tc.tile_pool(name="oTq", bufs=2) as oTq,
                tc.tile_pool(name="rdp", bufs=2) as rdp,
                tc.tile_pool(name="rdbcp", bufs=2) as rdbcp,
                tc.tile_pool(name="co", bufs=2) as cop,
            ):
                for q in range(NQB):
                    q0 = q * QB
                    oT_q = oTq.tile([NPART, PAIRS, QB], F32R, tag="oT")
                    for pr in range(PAIRS):
                        hA, hB = 2 * pr, 2 * pr + 1
                        avA = ps_av.tile([HD + 1, QB], F32, tag="av")
                        avB = ps_av.tile([HD + 1, QB], F32, tag="av")
                        for kt in range(NKT):
                            sp = ps_s.tile([NPART, 2 * QB], F32, tag="s")
                            nc.tensor.matmul(
                                out=sp[:, 0:QB],
                                lhsT=kall[0:HD, pr, kt * NPART : (kt + 1) * NPART],
                                rhs=qall[0:HD, pr, q0 : q0 + QB],
                                start=True, stop=True,
                            )
                            nc.tensor.matmul(
                                out=sp[:, QB : 2 * QB],
                                lhsT=kall[HD:NPART, pr, kt * NPART : (kt + 1) * NPART],
                                rhs=qall[HD:NPART, pr, q0 : q0 + QB],
                                start=True, stop=True,
                            )
                            pt = pTp.tile([NPART, 2 * QB], F32R, tag="pt")
                            nc.scalar.activation(
                                out=pt, in_=sp, func=AF.Exp,
                                bias=mb_sb[:, kt : kt + 1], scale=1.0 / 8.0,
                            )
                            nc.tensor.matmul(
                                out=avA,
                                lhsT=vs[:, kt, hA, :],
                                rhs=pt[:, 0:QB],
                                start=(kt == 0), stop=(kt == NKT - 1),
                            )
                            nc.tensor.matmul(
                                out=avB,
                                lhsT=vs[:, kt, hB, :],
                                rhs=pt[:, QB : 2 * QB],
                                start=(kt == 0), stop=(kt == NKT - 1),
                            )
                        for lohi, av in ((0, avA), (1, avB)):
                            rd0 = rdp.tile([1, QB], F32, tag="rd0")
                            nc.vector.tensor_copy(out=rd0, in_=av[HD : HD + 1, :])
                            rdr = rdp.tile([1, QB], F32, tag="rdr")
                            nc.vector.reciprocal_approx_fast(out=rdr, in_=rd0)
                            rdbc = rdbcp.tile([HD, QB], F32, tag="rdbc")
                            nc.gpsimd.partition_broadcast(rdbc, rdr)
                            nc.vector.tensor_tensor(
                                out=oT_q[lohi * HD : (lohi + 1) * HD, pr, :],
                                in0=av[0:HD, :], in1=rdbc, op=OP.mult,
                            )
                    # ---- phase C1 for this q block: out-proj + stats + spill ----
                    for j in range(4):
                        st = q * 4 + j
                        po_t = ps_s.tile([NPART, 2 * QB], F32, tag="s")
                        po = po_t[:, 0:DIM]
                        for pr2 in range(PAIRS):
                            nc.tensor.matmul(
                                out=po,
                                lhsT=oT_q[:, pr2, j * NPART : (j + 1) * NPART],
                                rhs=wo_sb[:, pr2, :],
                                start=(pr2 == 0), stop=False,
                            )
                        nc.tensor.matmul(out=po, lhsT=ones1, rhs=bout_sb, start=False, stop=True)
                        nc.vector.bn_stats(out=stats_sb[:, st, :], in_=po)
                        oc = cop.tile([NPART, DIM], F32, tag="oc")
                        nc.vector.tensor_copy(out=oc, in_=po)
                        nc.sync.dma_start(
                            out=d_oraw[st * NPART : (st + 1) * NPART, :], in_=oc
                        )

                # ================= phase C2: LN stats =================
                for st in range(ST):
                    nc.vector.bn_aggr(out=mv_sb[:, st, :], in_=stats_sb[:, st, :])
                nc.scalar.activation(
                    out=sd_sb, in_=mv_sb[:, :, 1], func=AF.Sqrt, bias=eps_sb,
                )
                nc.vector.reciprocal_approx_fast(out=rsig_sb, in_=sd_sb)

                # ================= phase C3: LN apply + ELU + store =================
                for st in range(ST):
                    ld = cop.tile([NPART, DIM], F32, tag="ld")
                    nc.sync.dma_start(
                        out=ld, in_=d_oraw[st * NPART : (st + 1) * NPART, :]
                    )
                    y = cop.tile([NPART, DIM], F32, tag="y")
                    nc.vector.tensor_scalar(
                        out=y, in0=ld,
                        scalar1=mv_sb[:, st, 0:1], scalar2=rsig_sb[:, st : st + 1],
                        op0=OP.subtract, op1=OP.mult,
                    )
                    if genln:
                        nc.vector.tensor_tensor(out=y, in0=y, in1=gbc, op=OP.mult)
                        nc.vector.tensor_tensor(out=y, in0=y, in1=bbc, op=OP.add)
                    tmin = tmp.tile([NPART, DIM], F32, tag="t0")
                    nc.vector.tensor_scalar_min(out=tmin, in0=y, scalar1=0.0)
                    te = tmp.tile([NPART, DIM], F32, tag="t1")
                    nc.scalar.activation(out=te, in_=tmin, func=AF.Exp)
                    tr = tmp.tile([NPART, DIM], F32, tag="t2")
                    nc.vector.tensor_scalar(
                        out=tr, in0=y, scalar1=0.0, scalar2=-1.0, op0=OP.max, op1=OP.add,
                    )
                    fin = cop.tile([NPART, DIM], F32, tag="fin")
                    nc.vector.tensor_tensor(out=fin, in0=te, in1=tr, op=OP.add)
                    nc.sync.dma_start(
                        out=d_out[st * NPART : (st + 1) * NPART, :], in_=fin
                    )

    nc.compile()
    return nc


def _install_trace_hook():
    try:
        import antenv  # noqa: F401
        from trn_agent_boot.trn_boot import _ntff_profile_via_ctypes

        hook = _ntff_profile_via_ctypes("/opt/axon/libaxon_pjrt.so")
        mod = types.ModuleType("antenv.axon_hooks")
        mod.get_axon_ntff_profile_hook = lambda: hook
        sys.modules["antenv.axon_hooks"] = mod
    except Exception:
        pass


def kernel(x, mask, Wqkv, bqkv, Wout, bout, ln_g, ln_b):
    global LAST_RESULT
    from concourse.bass_utils import run_bass_kernel_spmd

    per_core, shared, NKT, genln = _host_prep(
        x, mask, Wqkv, bqkv, Wout, bout, ln_g, ln_b
    )
    key = (NKT, genln)
    if key not in _PROG_CACHE:
        _PROG_CACHE[key] = _build_program(NKT, genln)
    nc = _PROG_CACHE[key]

    in_maps = [{**shared, **pc} for pc in per_core]
    if TRACE:
        _install_trace_hook()
    res = run_bass_kernel_spmd(
        nc, in_maps, list(range(B)), trace=TRACE, **TRACE_KWARGS
    )
    LAST_RESULT = res
    out = np.stack([res.results[i]["out"] for i in range(B)], axis=0)
    return out.astype(np.float32)


# revision 4
# speedup vs baseline: 1.2518x; 1.0993x over previous
"""Trainium2 Bass kernel for nn_MultiHeadAttention (B=8, S=2048, D=512, H=8).

Sharding: data-parallel over batch; core i computes batch element i end-to-end.
Key compaction: masked key positions contribute exactly zero after softmax, so
the host gathers only unmasked key rows (padded to a multiple of 128 with
entries whose mask bias of -30000 makes exp() underflow to zero).
"""

import sys
import types

for _p in ("/opt/trn_rl_repo", "/root/.axon_site"):
    if _p not in sys.path:
        sys.path.insert(0, _p)

import numpy as np

B, S, DIM, H, HD = 8, 2048, 512, 8, 64


def _bf16():
    import ml_dtypes

    return ml_dtypes.bfloat16
NPART = 128
PAIRS = H // 2  # head pairs (2 heads share a 128-partition tile)
ST = S // NPART  # 16 query/row tiles
QB = 512  # query block width for attention
NQB = S // QB  # 4
MASK_BIAS = -30000.0

TRACE = False
TRACE_KWARGS = {}
LAST_RESULT = None
_PROG_CACHE = {}


def _host_prep(x, mask, Wqkv, bqkv, Wout, bout, ln_g, ln_b):
    x = np.ascontiguousarray(x, dtype=np.float32)
    mask = np.asarray(mask).astype(bool)
    Wqkv = np.asarray(Wqkv, dtype=np.float32)
    bqkv = np.asarray(bqkv, dtype=np.float32)
    Wout = np.asarray(Wout, dtype=np.float32)
    bout = np.asarray(bout, dtype=np.float32)

    # torch-reshape-interleaved channel indices: j = 192*h + 3*d + c
    d_idx = np.arange(HD)
    perm_q = np.concatenate([192 * h + 3 * d_idx for h in range(H)])
    perm_k = perm_q + 1
    perm_v = perm_q + 2

    wqT = np.ascontiguousarray(Wqkv[perm_q, :].T)  # [512 d, 512 ch(h-major)]
    wkT = np.ascontiguousarray(Wqkv[perm_k, :].T)
    wvT = np.ascontiguousarray(Wqkv[perm_v, :].T)
    bq = np.ascontiguousarray(bqkv[perm_q].reshape(PAIRS, NPART))
    bk = np.ascontiguousarray(bqkv[perm_k].reshape(PAIRS, NPART))
    bv = np.ascontiguousarray(bqkv[perm_v].reshape(1, DIM))
    # Wout columns are already (h, d)-ordered; head-pair-major rows for out-proj rhs
    woT = np.ascontiguousarray(Wout.T)  # [512 c(h-major), 512 e]
    bout_r = np.ascontiguousarray(bout.reshape(1, DIM))

    genln = not (np.allclose(ln_g, 1.0) and np.allclose(ln_b, 0.0))

    keep = [np.flatnonzero(~mask[b]) for b in range(B)]
    nk_max = max(len(k) for k in keep)
    NK = max(NPART, ((nk_max + NPART - 1) // NPART) * NPART)
    NKT = NK // NPART

    per_core = []
    for b in range(B):
        kidx = np.zeros(NK, dtype=np.int64)
        kidx[: len(keep[b])] = keep[b]
        mb = np.full(NK, MASK_BIAS, dtype=np.float32)
        mb[: len(keep[b])] = 0.0
        per_core.append(
            dict(
                xq=np.ascontiguousarray(x[b].T),  # [512, 2048]
                xk=np.ascontiguousarray(x[b][kidx, :].T),  # [512, NK]
                mb=np.ascontiguousarray(mb.reshape(NKT, NPART).T),  # [128, NKT]
            )
        )

    shared = dict(
        wqT=wqT, wkT=wkT, wvT=wvT, bq=bq, bk=bk, bv=bv,
        woT=woT, bout=bout_r,
        ones=np.ones((1, NPART), dtype=np.float32),
        vones=np.ones((NPART, NKT, H), dtype=_bf16()),
    )
    if genln:
        shared["lng"] = np.ascontiguousarray(np.asarray(ln_g, np.float32).reshape(1, DIM))
        shared["lnb"] = np.ascontiguousarray(np.asarray(ln_b, np.float32).reshape(1, DIM))
    return per_core, shared, NKT, genln


def _build_program(NKT, genln):
    import concourse.bass as bass
    import concourse.tile as tile
    from concourse import mybir, bacc

    F32 = mybir.dt.float32
    F32R = mybir.dt.float32r
    BF16 = mybir.dt.bfloat16
    AF = mybir.ActivationFunctionType
    OP = mybir.AluOpType
    NK = NKT * NPART

    nc = bacc.Bacc("TRN2", target_bir_lowering=False, debug=False)

    d_xq = nc.dram_tensor("xq", [DIM, S], F32R, kind="ExternalInput").ap()
    d_xk = nc.dram_tensor("xk", [DIM, NK], F32R, kind="ExternalInput").ap()
    d_mb = nc.dram_tensor("mb", [NPART, NKT], F32, kind="ExternalInput").ap()
    d_wqT = nc.dram_tensor("wqT", [DIM, DIM], F32R, kind="ExternalInput").ap()
    d_wkT = nc.dram_tensor("wkT", [DIM, DIM], F32R, kind="ExternalInput").ap()
    d_wvT = nc.dram_tensor("wvT", [DIM, DIM], F32R, kind="ExternalInput").ap()
    d_bq = nc.dram_tensor("bq", [PAIRS, NPART], F32, kind="ExternalInput").ap()
    d_bk = nc.dram_tensor("bk", [PAIRS, NPART], F32, kind="ExternalInput").ap()
    d_bv = nc.dram_tensor("bv", [1, DIM], F32R, kind="ExternalInput").ap()
    d_woT = nc.dram_tensor("woT", [DIM, DIM], F32R, kind="ExternalInput").ap()
    d_bout = nc.dram_tensor("bout", [1, DIM], F32R, kind="ExternalInput").ap()
    d_ones = nc.dram_tensor("ones", [1, NPART], F32R, kind="ExternalInput").ap()
    d_vones = nc.dram_tensor("vones", [NPART, NKT, H], BF16, kind="ExternalInput").ap()
    if genln:
        d_lng = nc.dram_tensor("lng", [1, DIM], F32, kind="ExternalInput").ap()
        d_lnb = nc.dram_tensor("lnb", [1, DIM], F32, kind="ExternalInput").ap()
    d_out = nc.dram_tensor("out", [S, DIM], F32, kind="ExternalOutput").ap()
    d_oraw = nc.dram_tensor("oraw", [S, DIM], F32)  # pre-LN spill

    with tile.TileContext(nc) as tc:
        with (
            tc.tile_pool(name="persist", bufs=1) as pp,
            tc.tile_pool(name="ps_s", bufs=2, space="PSUM") as ps_s,
            tc.tile_pool(name="ps_av", bufs=4, space="PSUM") as ps_av,
            tc.tile_pool(name="tmp", bufs=3) as tmp,
        ):
            # ---- persistent tiles ----
            qall = pp.tile([NPART, PAIRS, S], BF16)
            kall = pp.tile([NPART, PAIRS, NK], BF16)
            v_sb = pp.tile([NPART, NKT, H * (HD + 1)], BF16)
            mb_sb = pp.tile([NPART, NKT], F32)
            bq_sb = pp.tile([NPART, PAIRS], F32)
            bk_sb = pp.tile([NPART, PAIRS], F32)
            bv_sb = pp.tile([1, DIM], F32R)
            bout_sb = pp.tile([1, DIM], F32R)
            ones1 = pp.tile([1, NPART], F32R)
            wo_sb = pp.tile([NPART, PAIRS, DIM], F32R)
            eps_sb = pp.tile([NPART, 1], F32)
            stats_sb = pp.tile([NPART, ST, 6], F32)
            mv_sb = pp.tile([NPART, ST, 2], F32)
            sd_sb = pp.tile([NPART, ST], F32)
            rsig_sb = pp.tile([NPART, ST], F32)

            nc.sync.dma_start(out=mb_sb, in_=d_mb)
            for p in range(PAIRS):
                nc.sync.dma_start(out=bq_sb[:, p : p + 1], in_=d_bq[p : p + 1, :].rearrange("a b -> b a"))
                nc.sync.dma_start(out=bk_sb[:, p : p + 1], in_=d_bk[p : p + 1, :].rearrange("a b -> b a"))
            nc.sync.dma_start(out=bv_sb, in_=d_bv)
            nc.sync.dma_start(out=bout_sb, in_=d_bout)
            nc.sync.dma_start(out=ones1, in_=d_ones)
            nc.sync.dma_start(out=wo_sb, in_=d_woT.rearrange("(pr p) e -> p pr e", p=NPART))
            nc.vector.memset(eps_sb, 1e-5)
            vs = v_sb.rearrange("p t (h c) -> p t h c", c=HD + 1)
            nc.sync.dma_start(out=vs[:, :, :, HD : HD + 1], in_=d_vones)
            if genln:
                g_row = pp.tile([1, DIM], F32)
                b_row = pp.tile([1, DIM], F32)
                gbc = pp.tile([NPART, DIM], F32)
                bbc = pp.tile([NPART, DIM], F32)
                nc.sync.dma_start(out=g_row, in_=d_lng)
                nc.sync.dma_start(out=b_row, in_=d_lnb)
                nc.gpsimd.partition_broadcast(gbc, g_row)
                nc.gpsimd.partition_broadcast(bbc, b_row)

            # ================= phase A1: Q/K projections =================
            with (
                tc.tile_pool(name="pA1w", bufs=1) as pA1w,
                tc.tile_pool(name="pAxq", bufs=1) as pAxq,
                tc.tile_pool(name="pAxk", bufs=1) as pAxk,
            ):
                wq_sb = pA1w.tile([NPART, 4, DIM], F32R)
                wk_sb = pA1w.tile([NPART, 4, DIM], F32R)
                xq_sb = pAxq.tile([NPART, 4, S], F32R)
                xk_sb = pAxk.tile([NPART, 4, NK], F32R)
                wq_v = d_wqT.rearrange("(kt p) c -> p kt c", p=NPART)
                wk_v = d_wkT.rearrange("(kt p) c -> p kt c", p=NPART)
                xq_v = d_xq.rearrange("(kt p) s -> p kt s", p=NPART)
                xk_v = d_xk.rearrange("(kt p) s -> p kt s", p=NPART)
                for kt in range(4):
                    nc.sync.dma_start(out=wq_sb[:, kt, :], in_=wq_v[:, kt, :])
                    nc.sync.dma_start(out=wk_sb[:, kt, :], in_=wk_v[:, kt, :])
                    nc.sync.dma_start(out=xq_sb[:, kt, :], in_=xq_v[:, kt, :])
                    nc.sync.dma_start(out=xk_sb[:, kt, :], in_=xk_v[:, kt, :])

                for p in range(PAIRS):
                    for n0 in range(0, S, QB):
                        ps = ps_s.tile([NPART, 2 * QB], F32, tag="s")
                        for kt in range(4):
                            nc.tensor.matmul(
                                out=ps[:, 0:QB],
                                lhsT=wq_sb[:, kt, p * NPART : (p + 1) * NPART],
                                rhs=xq_sb[:, kt, n0 : n0 + QB],
                                start=(kt == 0), stop=(kt == 3),
                            )
                        nc.vector.tensor_scalar_add(
                            out=qall[:, p, n0 : n0 + QB], in0=ps[:, 0:QB],
                            scalar1=bq_sb[:, p : p + 1],
                        )
                    for n0 in range(0, NK, QB):
                        n1 = min(n0 + QB, NK)
                        ps = ps_s.tile([NPART, 2 * QB], F32, tag="s")
                        for kt in range(4):
                            nc.tensor.matmul(
                                out=ps[:, 0 : n1 - n0],
                                lhsT=wk_sb[:, kt, p * NPART : (p + 1) * NPART],
                                rhs=xk_sb[:, kt, n0:n1],
                                start=(kt == 0), stop=(kt == 3),
                            )
                        nc.vector.tensor_scalar_add(
                            out=kall[:, p, n0:n1], in0=ps[:, 0 : n1 - n0],
                            scalar1=bk_sb[:, p : p + 1],
                        )

                # ============= phase A2: V projection + ELU =============
                with tc.tile_pool(name="pA2w", bufs=1) as pA2w:
                    wv_sb = pA2w.tile([NPART, 4, DIM], F32R)
                    wv_v = d_wvT.rearrange("(kt p) c -> p kt c", p=NPART)
                    for kt in range(4):
                        nc.sync.dma_start(out=wv_sb[:, kt, :], in_=wv_v[:, kt, :])
                    for st in range(NKT):
                        ps = ps_s.tile([NPART, 2 * QB], F32, tag="s")
                        pv = ps[:, 0:DIM]
                        for kt in range(4):
                            nc.tensor.matmul(
                                out=pv,
                                lhsT=xk_sb[:, kt, st * NPART : (st + 1) * NPART],
                                rhs=wv_sb[:, kt, :],
                                start=(kt == 0), stop=False,
                            )
                        nc.tensor.matmul(out=pv, lhsT=ones1, rhs=bv_sb, start=False, stop=True)
                        # elu(x) = exp(min(x,0)) + max(x,0) - 1
                        tmin = tmp.tile([NPART, DIM], F32, tag="t0")
                        nc.vector.tensor_scalar_min(out=tmin, in0=pv, scalar1=0.0)
                        te = tmp.tile([NPART, DIM], F32, tag="t1")
                        nc.scalar.activation(out=te, in_=tmin, func=AF.Exp)
                        tr = tmp.tile([NPART, DIM], F32, tag="t2")
                        nc.vector.tensor_scalar(
                            out=tr, in0=pv, scalar1=0.0, scalar2=-1.0,
                            op0=OP.max, op1=OP.add,
                        )
                        nc.vector.tensor_tensor(
                            out=vs[:, st, :, 0:HD],
                            in0=te.rearrange("p (h c) -> p h c", c=HD),
                            in1=tr.rearrange("p (h c) -> p h c", c=HD),
                            op=OP.add,
                        )

            # ================= phase B: attention + out-proj =================
            with (
                tc.tile_pool(name="pT", bufs=3) as pTp,
                tc.tile_pool(name="oTq", bufs=2) as oTq,
                tc.tile_pool(name="rdp", bufs=2) as rdp,
                tc.tile_pool(name="rdbcp", bufs=2) as rdbcp,
                tc.tile_pool(name="co", bufs=2) as cop,
            ):
                for q in range(NQB):
                    q0 = q * QB
                    oT_q = oTq.tile([NPART, PAIRS, QB], F32R, tag="oT")
                    for pr in range(PAIRS):
                        hA, hB = 2 * pr, 2 * pr + 1
                        avA = ps_av.tile([HD + 1, QB], F32, tag="av")
                        avB = ps_av.tile([HD + 1, QB], F32, tag="av")
                        for kt in range(NKT):
                            sp = ps_s.tile([NPART, 2 * QB], F32, tag="s")
                            nc.tensor.matmul(
                                out=sp[:, 0:QB],
                                lhsT=kall[0:HD, pr, kt * NPART : (kt + 1) * NPART],
                                rhs=qall[0:HD, pr, q0 : q0 + QB],
                                start=True, stop=True,
                            )
                            nc.tensor.matmul(
                                out=sp[:, QB : 2 * QB],
                                lhsT=kall[HD:NPART, pr, kt * NPART : (kt + 1) * NPART],
                                rhs=qall[HD:NPART, pr, q0 : q0 + QB],
                                start=True, stop=True,
                            )
                            pt = pTp.tile([NPART, 2 * QB], BF16, tag="pt")
                            nc.scalar.activation(
                                out=pt, in_=sp, func=AF.Exp,
                                bias=mb_sb[:, kt : kt + 1], scale=1.0 / 8.0,
                            )
                            nc.tensor.matmul(
                                out=avA,
                                lhsT=vs[:, kt, hA, :],
                                rhs=pt[:, 0:QB],
                                start=(kt == 0), stop=(kt == NKT - 1),
                            )
                            nc.tensor.matmul(
                                out=avB,
                                lhsT=vs[:, kt, hB, :],
                                rhs=pt[:, QB : 2 * QB],
                                start=(kt == 0), stop=(kt == NKT - 1),
                            )
                        for lohi, av in ((0, avA), (1, avB)):
                            rd0 = rdp.tile([1, QB], F32, tag="rd0")
                            nc.vector.tensor_copy(out=rd0, in_=av[HD : HD + 1, :])
                            rdr = rdp.tile([1, QB], F32, tag="rdr")
                            nc.vector.reciprocal_approx_fast(out=rdr, in_=rd0)
                            rdbc = rdbcp.tile([HD, QB], F32, tag="rdbc")
                            nc.gpsimd.partition_broadcast(rdbc, rdr)
                            nc.vector.tensor_tensor(
                                out=oT_q[lohi * HD : (lohi + 1) * HD, pr, :],
                                in0=av[0:HD, :], in1=rdbc, op=OP.mult,
                            )
                    # ---- phase C1 for this q block: out-proj + stats + spill ----
                    for j in range(4):
                        st = q * 4 + j
                        po_t = ps_s.tile([NPART, 2 * QB], F32, tag="s")
                        po = po_t[:, 0:DIM]
                        for pr2 in range(PAIRS):
                            nc.tensor.matmul(
                                out=po,
                                lhsT=oT_q[:, pr2, j * NPART : (j + 1) * NPART],
                                rhs=wo_sb[:, pr2, :],
                                start=(pr2 == 0), stop=False,
                            )
                        nc.tensor.matmul(out=po, lhsT=ones1, rhs=bout_sb, start=False, stop=True)
                        nc.vector.bn_stats(out=stats_sb[:, st, :], in_=po)
                        oc = cop.tile([NPART, DIM], F32, tag="oc")
                        nc.vector.tensor_copy(out=oc, in_=po)
                        nc.sync.dma_start(
                            out=d_oraw[st * NPART : (st + 1) * NPART, :], in_=oc
                        )

                # ================= phase C2: LN stats =================
                for st in range(ST):
                    nc.vector.bn_aggr(out=mv_sb[:, st, :], in_=stats_sb[:, st, :])
                nc.scalar.activation(
                    out=sd_sb, in_=mv_sb[:, :, 1], func=AF.Sqrt, bias=eps_sb,
                )
                nc.vector.reciprocal_approx_fast(out=rsig_sb, in_=sd_sb)

                # ================= phase C3: LN apply + ELU + store =================
                for st in range(ST):
                    ld = cop.tile([NPART, DIM], F32, tag="ld")
                    nc.sync.dma_start(
                        out=ld, in_=d_oraw[st * NPART : (st + 1) * NPART, :]
                    )
                    y = cop.tile([NPART, DIM], F32, tag="y")
                    nc.vector.tensor_scalar(
                        out=y, in0=ld,
                        scalar1=mv_sb[:, st, 0:1], scalar2=rsig_sb[:, st : st + 1],
                        op0=OP.subtract, op1=OP.mult,
                    )
                    if genln:
                        nc.vector.tensor_tensor(out=y, in0=y, in1=gbc, op=OP.mult)
                        nc.vector.tensor_tensor(out=y, in0=y, in1=bbc, op=OP.add)
                    tmin = tmp.tile([NPART, DIM], F32, tag="t0")
                    nc.vector.tensor_scalar_min(out=tmin, in0=y, scalar1=0.0)
                    te = tmp.tile([NPART, DIM], F32, tag="t1")
                    nc.scalar.activation(out=te, in_=tmin, func=AF.Exp)
                    tr = tmp.tile([NPART, DIM], F32, tag="t2")
                    nc.vector.tensor_scalar(
                        out=tr, in0=y, scalar1=0.0, scalar2=-1.0, op0=OP.max, op1=OP.add,
                    )
                    fin = cop.tile([NPART, DIM], F32, tag="fin")
                    nc.vector.tensor_tensor(out=fin, in0=te, in1=tr, op=OP.add)
                    nc.sync.dma_start(
                        out=d_out[st * NPART : (st + 1) * NPART, :], in_=fin
                    )

    nc.compile()
    return nc


def _install_trace_hook():
    try:
        import antenv  # noqa: F401
        from trn_agent_boot.trn_boot import _ntff_profile_via_ctypes

        hook = _ntff_profile_via_ctypes("/opt/axon/libaxon_pjrt.so")
        mod = types.ModuleType("antenv.axon_hooks")
        mod.get_axon_ntff_profile_hook = lambda: hook
        sys.modules["antenv.axon_hooks"] = mod
    except Exception:
        pass


def kernel(x, mask, Wqkv, bqkv, Wout, bout, ln_g, ln_b):
    global LAST_RESULT
    from concourse.bass_utils import run_bass_kernel_spmd

    per_core, shared, NKT, genln = _host_prep(
        x, mask, Wqkv, bqkv, Wout, bout, ln_g, ln_b
    )
    key = (NKT, genln)
    if key not in _PROG_CACHE:
        _PROG_CACHE[key] = _build_program(NKT, genln)
    nc = _PROG_CACHE[key]

    in_maps = [{**shared, **pc} for pc in per_core]
    if TRACE:
        _install_trace_hook()
    res = run_bass_kernel_spmd(
        nc, in_maps, list(range(B)), trace=TRACE, **TRACE_KWARGS
    )
    LAST_RESULT = res
    out = np.stack([res.results[i]["out"] for i in range(B)], axis=0)
    return out.astype(np.float32)


# revision 8
# speedup vs baseline: 1.4641x; 1.1696x over previous
"""Trainium2 Bass kernel for nn_MultiHeadAttention (B=8, S=2048, D=512, H=8).

Sharding: data-parallel over batch; core i computes batch element i end-to-end.
Key compaction: masked key positions contribute exactly zero after softmax, so
the host gathers only unmasked key rows (padded to a multiple of 128 with
entries whose mask bias of -30000 makes exp() underflow to zero).
"""

import sys
import types

for _p in ("/opt/trn_rl_repo", "/root/.axon_site"):
    if _p not in sys.path:
        sys.path.insert(0, _p)

import numpy as np

B, S, DIM, H, HD = 8, 2048, 512, 8, 64


def _bf16():
    import ml_dtypes

    return ml_dtypes.bfloat16
NPART = 128
PAIRS = H // 2  # head pairs (2 heads share a 128-partition tile)
ST = S // NPART  # 16 query/row tiles
QB = 512  # query block width for attention
NQB = S // QB  # 4
MASK_BIAS = -30000.0

TRACE = False
TRACE_KWARGS = {}
LAST_RESULT = None
_PROG_CACHE = {}


def _host_prep(x, mask, Wqkv, bqkv, Wout, bout, ln_g, ln_b):
    x = np.ascontiguousarray(x, dtype=np.float32)
    mask = np.asarray(mask).astype(bool)
    Wqkv = np.asarray(Wqkv, dtype=np.float32)
    bqkv = np.asarray(bqkv, dtype=np.float32)
    Wout = np.asarray(Wout, dtype=np.float32)
    bout = np.asarray(bout, dtype=np.float32)

    # torch-reshape-interleaved channel indices: j = 192*h + 3*d + c
    d_idx = np.arange(HD)
    perm_q = np.concatenate([192 * h + 3 * d_idx for h in range(H)])
    perm_k = perm_q + 1
    perm_v = perm_q + 2

    bf16 = _bf16()
    wqT = np.ascontiguousarray(Wqkv[perm_q, :].T).astype(bf16)  # [512 d, 512 ch(h-major)]
    wkT = np.ascontiguousarray(Wqkv[perm_k, :].T).astype(bf16)
    wvT = np.ascontiguousarray(Wqkv[perm_v, :].T).astype(bf16)
    bq = np.ascontiguousarray(bqkv[perm_q].reshape(PAIRS, NPART))
    bk = np.ascontiguousarray(bqkv[perm_k].reshape(PAIRS, NPART))
    bv = np.ascontiguousarray(bqkv[perm_v].reshape(1, DIM))
    # Wout columns are already (h, d)-ordered; head-pair-major rows for out-proj rhs
    woT = np.ascontiguousarray(Wout.T)  # [512 c(h-major), 512 e]
    bout_r = np.ascontiguousarray(bout.reshape(1, DIM))

    genln = not (np.allclose(ln_g, 1.0) and np.allclose(ln_b, 0.0))

    keep = [np.flatnonzero(~mask[b]) for b in range(B)]
    nk_max = max(len(k) for k in keep)
    NK = max(NPART, ((nk_max + NPART - 1) // NPART) * NPART)
    NKT = NK // NPART

    per_core = []
    for b in range(B):
        kidx = np.zeros(NK, dtype=np.int64)
        kidx[: len(keep[b])] = keep[b]
        mb = np.full(NK, MASK_BIAS, dtype=np.float32)
        mb[: len(keep[b])] = 0.0
        per_core.append(
            dict(
                xq=np.ascontiguousarray(x[b].T).astype(bf16),  # [512, 2048]
                xk=np.ascontiguousarray(x[b][kidx, :].T).astype(bf16),  # [512, NK]
                mb=np.ascontiguousarray(mb.reshape(NKT, NPART).T),  # [128, NKT]
            )
        )

    shared = dict(
        wqT=wqT, wkT=wkT, wvT=wvT, bq=bq, bk=bk, bv=bv,
        woT=woT, bout=bout_r,
        ones=np.ones((1, NPART), dtype=np.float32),
        vones=np.ones((NPART, NKT, H), dtype=_bf16()),
    )
    if genln:
        shared["lng"] = np.ascontiguousarray(np.asarray(ln_g, np.float32).reshape(1, DIM))
        shared["lnb"] = np.ascontiguousarray(np.asarray(ln_b, np.float32).reshape(1, DIM))
    return per_core, shared, NKT, genln


def _build_program(NKT, genln):
    import concourse.bass as bass
    import concourse.tile as tile
    from concourse import mybir, bacc

    F32 = mybir.dt.float32
    F32R = mybir.dt.float32r
    BF16 = mybir.dt.bfloat16
    AF = mybir.ActivationFunctionType
    OP = mybir.AluOpType
    NK = NKT * NPART

    nc = bacc.Bacc("TRN2", target_bir_lowering=False, debug=False)

    d_xq = nc.dram_tensor("xq", [DIM, S], BF16, kind="ExternalInput").ap()
    d_xk = nc.dram_tensor("xk", [DIM, NK], BF16, kind="ExternalInput").ap()
    d_mb = nc.dram_tensor("mb", [NPART, NKT], F32, kind="ExternalInput").ap()
    d_wqT = nc.dram_tensor("wqT", [DIM, DIM], BF16, kind="ExternalInput").ap()
    d_wkT = nc.dram_tensor("wkT", [DIM, DIM], BF16, kind="ExternalInput").ap()
    d_wvT = nc.dram_tensor("wvT", [DIM, DIM], BF16, kind="ExternalInput").ap()
    d_bq = nc.dram_tensor("bq", [PAIRS, NPART], F32, kind="ExternalInput").ap()
    d_bk = nc.dram_tensor("bk", [PAIRS, NPART], F32, kind="ExternalInput").ap()
    d_bv = nc.dram_tensor("bv", [1, DIM], F32R, kind="ExternalInput").ap()
    d_woT = nc.dram_tensor("woT", [DIM, DIM], F32R, kind="ExternalInput").ap()
    d_bout = nc.dram_tensor("bout", [1, DIM], F32R, kind="ExternalInput").ap()
    d_ones = nc.dram_tensor("ones", [1, NPART], F32R, kind="ExternalInput").ap()
    d_vones = nc.dram_tensor("vones", [NPART, NKT, H], BF16, kind="ExternalInput").ap()
    if genln:
        d_lng = nc.dram_tensor("lng", [1, DIM], F32, kind="ExternalInput").ap()
        d_lnb = nc.dram_tensor("lnb", [1, DIM], F32, kind="ExternalInput").ap()
    d_out = nc.dram_tensor("out", [S, DIM], F32, kind="ExternalOutput").ap()

    with tile.TileContext(nc) as tc:
        with (
            tc.tile_pool(name="persist", bufs=1) as pp,
            tc.tile_pool(name="ps_s", bufs=2, space="PSUM") as ps_s,
            tc.tile_pool(name="ps_av", bufs=4, space="PSUM") as ps_av,
            tc.tile_pool(name="tmp", bufs=3) as tmp,
        ):
            # ---- persistent tiles ----
            qall = pp.tile([NPART, PAIRS, S], BF16)
            kall = pp.tile([NPART, PAIRS, NK], BF16)
            v_sb = pp.tile([NPART, NKT, H * (HD + 1)], BF16)
            mb_sb = pp.tile([NPART, NKT], F32)
            bq_sb = pp.tile([NPART, PAIRS], F32)
            bk_sb = pp.tile([NPART, PAIRS], F32)
            bv_sb = pp.tile([1, DIM], F32R)
            bout_sb = pp.tile([1, DIM], F32R)
            ones1 = pp.tile([1, NPART], F32R)
            wo_sb = pp.tile([NPART, PAIRS, DIM], F32R)
            eps_sb = pp.tile([NPART, 1], F32)
            stats_sb = pp.tile([NPART, ST, 6], F32)
            mv_sb = pp.tile([NPART, ST, 2], F32)
            sd_sb = pp.tile([NPART, ST], F32)
            rsig_sb = pp.tile([NPART, ST], F32)

            vs = v_sb.rearrange("p t (h c) -> p t h c", c=HD + 1)
            if genln:
                g_row = pp.tile([1, DIM], F32)
                b_row = pp.tile([1, DIM], F32)
                gbc = pp.tile([NPART, DIM], F32)
                bbc = pp.tile([NPART, DIM], F32)
                nc.sync.dma_start(out=g_row, in_=d_lng)
                nc.sync.dma_start(out=b_row, in_=d_lnb)
                nc.gpsimd.partition_broadcast(gbc, g_row)
                nc.gpsimd.partition_broadcast(bbc, b_row)

            # ================= phase A1: Q/K projections =================
            with (
                tc.tile_pool(name="pA1w", bufs=1) as pA1w,
                tc.tile_pool(name="pAxq", bufs=1) as pAxq,
                tc.tile_pool(name="pAxk", bufs=1) as pAxk,
            ):
                wq_sb = pA1w.tile([NPART, 4, DIM], BF16)
                wk_sb = pA1w.tile([NPART, 4, DIM], BF16)
                xq_sb = pAxq.tile([NPART, 4, S], BF16)
                xk_sb = pAxk.tile([NPART, 4, NK], BF16)
                wq_v = d_wqT.rearrange("(kt p) c -> p kt c", p=NPART)
                wk_v = d_wkT.rearrange("(kt p) c -> p kt c", p=NPART)
                xq_v = d_xq.rearrange("(kt p) s -> p kt s", p=NPART)
                xk_v = d_xk.rearrange("(kt p) s -> p kt s", p=NPART)
                for kt in range(4):
                    nc.sync.dma_start(out=wq_sb[:, kt, :], in_=wq_v[:, kt, :])
                    nc.sync.dma_start(out=wk_sb[:, kt, :], in_=wk_v[:, kt, :])
                    nc.sync.dma_start(out=xq_sb[:, kt, :], in_=xq_v[:, kt, :])
                    nc.sync.dma_start(out=xk_sb[:, kt, :], in_=xk_v[:, kt, :])

                nc.sync.dma_start(out=mb_sb, in_=d_mb)
                for p in range(PAIRS):
                    nc.sync.dma_start(out=bq_sb[:, p : p + 1], in_=d_bq[p : p + 1, :].rearrange("a b -> b a"))
                    nc.sync.dma_start(out=bk_sb[:, p : p + 1], in_=d_bk[p : p + 1, :].rearrange("a b -> b a"))
                nc.sync.dma_start(out=bv_sb, in_=d_bv)
                nc.sync.dma_start(out=bout_sb, in_=d_bout)
                nc.sync.dma_start(out=ones1, in_=d_ones)
                nc.sync.dma_start(out=wo_sb, in_=d_woT.rearrange("(pr p) e -> p pr e", p=NPART))
                nc.vector.memset(eps_sb, 1e-5)
                nc.sync.dma_start(out=vs[:, :, :, HD : HD + 1], in_=d_vones)

                for p in range(PAIRS):
                    for n0 in range(0, S, QB):
                        ps = ps_s.tile([NPART, 2 * QB], F32, tag="s")
                        for kt in range(4):
                            nc.tensor.matmul(
                                out=ps[:, 0:QB],
                                lhsT=wq_sb[:, kt, p * NPART : (p + 1) * NPART],
                                rhs=xq_sb[:, kt, n0 : n0 + QB],
                                start=(kt == 0), stop=(kt == 3),
                            )
                        nc.vector.tensor_scalar_add(
                            out=qall[:, p, n0 : n0 + QB], in0=ps[:, 0:QB],
                            scalar1=bq_sb[:, p : p + 1],
                        )
                    for n0 in range(0, NK, QB):
                        n1 = min(n0 + QB, NK)
                        ps = ps_s.tile([NPART, 2 * QB], F32, tag="s")
                        for kt in range(4):
                            nc.tensor.matmul(
                                out=ps[:, 0 : n1 - n0],
                                lhsT=wk_sb[:, kt, p * NPART : (p + 1) * NPART],
                                rhs=xk_sb[:, kt, n0:n1],
                                start=(kt == 0), stop=(kt == 3),
                            )
                        nc.vector.tensor_scalar_add(
                            out=kall[:, p, n0:n1], in0=ps[:, 0 : n1 - n0],
                            scalar1=bk_sb[:, p : p + 1],
                        )

                # ============= phase A2: V projection + ELU =============
                with tc.tile_pool(name="pA2w", bufs=1) as pA2w:
                    wv_sb = pA2w.tile([NPART, 4, DIM], BF16)
                    wv_v = d_wvT.rearrange("(kt p) c -> p kt c", p=NPART)
                    for kt in range(4):
                        nc.sync.dma_start(out=wv_sb[:, kt, :], in_=wv_v[:, kt, :])
                    for st in range(NKT):
                        ps = ps_s.tile([NPART, 2 * QB], F32, tag="s")
                        pv = ps[:, 0:DIM]
                        for kt in range(4):
                            nc.tensor.matmul(
                                out=pv,
                                lhsT=xk_sb[:, kt, st * NPART : (st + 1) * NPART],
                                rhs=wv_sb[:, kt, :],
                                start=(kt == 0), stop=False,
                            )
                        nc.tensor.matmul(out=pv, lhsT=ones1, rhs=bv_sb, start=False, stop=True)
                        # elu(x) = exp(min(x,0)) + max(x,0) - 1
                        tmin = tmp.tile([NPART, DIM], F32, tag="t0")
                        nc.vector.tensor_scalar_min(out=tmin, in0=pv, scalar1=0.0)
                        te = tmp.tile([NPART, DIM], F32, tag="t1")
                        nc.scalar.activation(out=te, in_=tmin, func=AF.Exp)
                        tr = tmp.tile([NPART, DIM], F32, tag="t2")
                        nc.vector.tensor_scalar(
                            out=tr, in0=pv, scalar1=0.0, scalar2=-1.0,
                            op0=OP.max, op1=OP.add,
                        )
                        nc.vector.tensor_tensor(
                            out=vs[:, st, :, 0:HD],
                            in0=te.rearrange("p (h c) -> p h c", c=HD),
                            in1=tr.rearrange("p (h c) -> p h c", c=HD),
                            op=OP.add,
                        )

            # ========== phase B + C, software-pipelined emission ==========
            with (
                tc.tile_pool(name="pT", bufs=4) as pTp,
                tc.tile_pool(name="oTq", bufs=2) as oTq,
                tc.tile_pool(name="rdp", bufs=3) as rdp,
                tc.tile_pool(name="rdbcp", bufs=3) as rdbcp,
                tc.tile_pool(name="poc", bufs=6) as poc,
                tc.tile_pool(name="pyf", bufs=3) as pyf,
            ):
                oT_tiles = {}
                oc_tiles = {}

                def emit_pair(q, pr):
                    q0 = q * QB
                    oT_q = oT_tiles[q]
                    hA, hB = 2 * pr, 2 * pr + 1
                    avA = ps_av.tile([HD + 1, QB], F32, tag="av")
                    avB = ps_av.tile([HD + 1, QB], F32, tag="av")
                    for kt in range(NKT):
                        sp = ps_s.tile([NPART, 2 * QB], F32, tag="s")
                        nc.tensor.matmul(
                            out=sp[:, 0:QB],
                            lhsT=kall[0:HD, pr, kt * NPART : (kt + 1) * NPART],
                            rhs=qall[0:HD, pr, q0 : q0 + QB],
                            start=True, stop=True,
                        )
                        nc.tensor.matmul(
                            out=sp[:, QB : 2 * QB],
                            lhsT=kall[HD:NPART, pr, kt * NPART : (kt + 1) * NPART],
                            rhs=qall[HD:NPART, pr, q0 : q0 + QB],
                            start=True, stop=True,
                        )
                        pt = pTp.tile([NPART, 2 * QB], BF16, tag="pt")
                        nc.scalar.activation(
                            out=pt, in_=sp, func=AF.Exp,
                            bias=mb_sb[:, kt : kt + 1], scale=1.0 / 8.0,
                        )
                        nc.tensor.matmul(
                            out=avA, lhsT=vs[:, kt, hA, :], rhs=pt[:, 0:QB],
                            start=(kt == 0), stop=(kt == NKT - 1),
                        )
                        nc.tensor.matmul(
                            out=avB, lhsT=vs[:, kt, hB, :], rhs=pt[:, QB : 2 * QB],
                            start=(kt == 0), stop=(kt == NKT - 1),
                        )
                    for lohi, av in ((0, avA), (1, avB)):
                        rd0 = rdp.tile([1, QB], F32, tag="rd0")
                        nc.vector.tensor_copy(out=rd0, in_=av[HD : HD + 1, :])
                        rdr = rdp.tile([1, QB], F32, tag="rdr")
                        nc.vector.reciprocal_approx_fast(out=rdr, in_=rd0)
                        rdbc = rdbcp.tile([HD, QB], F32, tag="rdbc")
                        nc.gpsimd.partition_broadcast(rdbc, rdr)
                        nc.vector.tensor_tensor(
                            out=oT_q[lohi * HD : (lohi + 1) * HD, pr, :],
                            in0=av[0:HD, :], in1=rdbc, op=OP.mult,
                        )

                def emit_c1(q):
                    oT_q = oT_tiles.pop(q)
                    for j in range(4):
                        st = q * 4 + j
                        po_t = ps_s.tile([NPART, 2 * QB], F32, tag="s")
                        po = po_t[:, 0:DIM]
                        for pr2 in range(PAIRS):
                            nc.tensor.matmul(
                                out=po,
                                lhsT=oT_q[:, pr2, j * NPART : (j + 1) * NPART],
                                rhs=wo_sb[:, pr2, :],
                                start=(pr2 == 0), stop=False,
                            )
                        nc.tensor.matmul(out=po, lhsT=ones1, rhs=bout_sb, start=False, stop=True)
                        nc.vector.bn_stats(out=stats_sb[:, st, :], in_=po)
                        oc = poc.tile([NPART, DIM], F32, tag="oc")
                        nc.vector.tensor_copy(out=oc, in_=po)
                        oc_tiles[st] = oc

                def emit_c2(q):
                    for j in range(4):
                        st = q * 4 + j
                        nc.vector.bn_aggr(out=mv_sb[:, st, :], in_=stats_sb[:, st, :])
                    # rsig = exp(-0.5 * ln(var + eps)) — Ln/Exp share one ACT table set
                    nc.scalar.activation(
                        out=sd_sb[:, q * 4 : q * 4 + 4],
                        in_=mv_sb[:, q * 4 : q * 4 + 4, 1],
                        func=AF.Ln, bias=eps_sb,
                    )
                    nc.scalar.activation(
                        out=rsig_sb[:, q * 4 : q * 4 + 4],
                        in_=sd_sb[:, q * 4 : q * 4 + 4],
                        func=AF.Exp, scale=-0.5,
                    )

                def emit_c3(q):
                    for j in range(4):
                        st = q * 4 + j
                        oc = oc_tiles.pop(st)
                        y = pyf.tile([NPART, DIM], F32, tag="y")
                        nc.vector.tensor_scalar(
                            out=y, in0=oc,
                            scalar1=mv_sb[:, st, 0:1], scalar2=rsig_sb[:, st : st + 1],
                            op0=OP.subtract, op1=OP.mult,
                        )
                        if genln:
                            nc.vector.tensor_tensor(out=y, in0=y, in1=gbc, op=OP.mult)
                            nc.vector.tensor_tensor(out=y, in0=y, in1=bbc, op=OP.add)
                        tmin = tmp.tile([NPART, DIM], F32, tag="t0")
                        nc.vector.tensor_scalar_min(out=tmin, in0=y, scalar1=0.0)
                        te = tmp.tile([NPART, DIM], F32, tag="t1")
                        nc.scalar.activation(out=te, in_=tmin, func=AF.Exp)
                        tr = tmp.tile([NPART, DIM], F32, tag="t2")
                        nc.vector.tensor_scalar(
                            out=tr, in0=y, scalar1=0.0, scalar2=-1.0, op0=OP.max, op1=OP.add,
                        )
                        fin = pyf.tile([NPART, DIM], F32, tag="fin")
                        nc.vector.tensor_tensor(out=fin, in0=te, in1=tr, op=OP.add)
                        nc.sync.dma_start(
                            out=d_out[st * NPART : (st + 1) * NPART, :], in_=fin
                        )

                for q in range(NQB):
                    oT_tiles[q] = oTq.tile([NPART, PAIRS, QB], F32R, tag="oT", name="oTq_t")
                    for pr in range(PAIRS):
                        emit_pair(q, pr)
                        if q >= 1:
                            if pr == 0:
                                emit_c1(q - 1)
                            elif pr == 1:
                                emit_c2(q - 1)
                            elif pr == 2:
                                emit_c3(q - 1)
                emit_c1(NQB - 1)
                emit_c2(NQB - 1)
                emit_c3(NQB - 1)

    nc.compile()
    return nc


def _install_trace_hook():
    try:
        import antenv  # noqa: F401
        from trn_agent_boot.trn_boot import _ntff_profile_via_ctypes

        hook = _ntff_profile_via_ctypes("/opt/axon/libaxon_pjrt.so")
        mod = types.ModuleType("antenv.axon_hooks")
        mod.get_axon_ntff_profile_hook = lambda: hook
        sys.modules["antenv.axon_hooks"] = mod
    except Exception:
        pass


def kernel(x, mask, Wqkv, bqkv, Wout, bout, ln_g, ln_b):
    global LAST_RESULT
    from concourse.bass_utils import run_bass_kernel_spmd

    per_core, shared, NKT, genln = _host_prep(
        x, mask, Wqkv, bqkv, Wout, bout, ln_g, ln_b
    )
    key = (NKT, genln)
    if key not in _PROG_CACHE:
        _PROG_CACHE[key] = _build_program(NKT, genln)
    nc = _PROG_CACHE[key]

    in_maps = [{**shared, **pc} for pc in per_core]
    if TRACE:
        _install_trace_hook()
    res = run_bass_kernel_spmd(
        nc, in_maps, list(range(B)), trace=TRACE, **TRACE_KWARGS
    )
    LAST_RESULT = res
    out = np.stack([res.results[i]["out"] for i in range(B)], axis=0)
    return out.astype(np.float32)
